# revision 1
# baseline (speedup 1.0000x reference)
"""Trainium2 Bass kernel for nn_Block_with_lora (dense transformer block).

Sharding: 8 cores = 4 batches x 2 token-parity shards (stride-2 over T).
Each core computes its 512 query tokens end-to-end (no collectives);
K/V projections over all 1024 tokens are computed per-core (uniform SPMD
program; all batch/parity dependence lives in the per-core input data).

Layout: all activations transposed [C, T] (host transposes I/O), so every
projection is a natural PE matmul. Attention uses S^T = K^T.T @ Q^T tiles
[tk, tq]; softmax denominator rides the AV matmul as an extra ones-column
of V; masking = additive diagonal band (DVE) + rectangle memsets (GPSIMD).
"""

import sys

sys.path.insert(0, "/opt/trn_rl_repo")

import numpy as np
import ml_dtypes
from contextlib import ExitStack

BF = ml_dtypes.bfloat16

C = 1024
H = 16
DH = 64
R = 16
SCALE = 1.0 / R
T = 1024
TQ = 512
NT = 8  # C / 128
EPS = 1e-5
NCORES = 8

_PROG = None


def _build_program():
    import concourse.bass as bass
    import concourse.tile as tile
    from concourse import mybir, bacc

    f32 = mybir.dt.float32
    bf16 = mybir.dt.bfloat16
    AF = mybir.ActivationFunctionType
    AL = mybir.AluOpType

    nc = bacc.Bacc("TRN2", target_bir_lowering=False, debug=False)

    def din(name, shape, dt=f32):
        return nc.dram_tensor(name, shape, dt, kind="ExternalInput").ap()

    xT_d = din("xT", [C, T])
    xqT_d = din("xqT", [C, TQ])
    fT_d = din("fT", [C, T])
    band_d = din("band", [128, 64])

    w_d = {}
    for n in ["wq", "wk", "wv", "wsp", "wcq", "wck", "wcv", "wcp"]:
        w_d[n] = din(n, [C, C], bf16)
    w_d["wfc"] = din("wfc", [C, 4 * C], bf16)
    w_d["wpr"] = din("wpr", [4 * C, C], bf16)
    a_d = {n: din(n, [C, R], bf16) for n in ["a_sa", "a_sp", "a_cq", "a_ck", "a_cp"]}
    b_d = {
        n: din(n, [R, C], bf16)
        for n in ["b_saq", "b_sak", "b_sav", "b_sp", "b_cq", "b_ckk", "b_ckv", "b_cp"]
    }
    bias_d = {
        n: din(n, [C], f32)
        for n in ["bq", "bk", "bsp", "bcq", "bck", "bcp", "bpr", "g1", "b1", "g2", "b2"]
    }
    bias_d["bfc"] = din("bfc", [4 * C], f32)
    bvrow_d = din("bv_row", [1, C], bf16)
    bckrow_d = din("bck_row", [1, C], bf16)
    sel_d = din("sel", [NT, R, 128], f32)
    bcvrow_d = din("bcv_row", [1, C], bf16)

    outT_d = nc.dram_tensor("outT", [C, TQ], f32, kind="ExternalOutput").ap()

    with tile.TileContext(nc) as tc, ExitStack() as ctx:

        def pool(name, bufs, space=None):
            kw = dict(name=name, bufs=bufs)
            if space:
                kw["space"] = space
            return ctx.enter_context(tc.tile_pool(**kw))

        # SBUF pools (budget ~181KB/partition of 192)
        big32 = pool("big32", 2)        # [128,1024] f32: x/f stream + LN temps
        acts = pool("acts", 8)          # [128,1024] bf16: lnb then fb
        lnsm = pool("lnsm", 8)          # [128,512] bf16: lnown -> ln1b -> ln2
        qpool = pool("qpool", 8)        # [128,512] bf16: qT -> q2T
        kpool = pool("kpool", 8)        # [128,1024] bf16: kT
        k2pool = pool("k2pool", 8)      # [128,1024] bf16: k2T (separate: overlaps attn)
        vpool = pool("vpool", 8)        # [128,1040] bf16: V -> V2
        opool = pool("opool", 8)        # [128,512] bf16: oT -> o2T
        rpool = pool("rpool", 8)        # [128,512] f32: residual (persist)
        mpool = pool("mpool", 32)       # [128,256] bf16: MLP hidden (per t-half)
        wpool = pool("wpool", 10)        # [128,512] bf16: weight chunks
        epool = pool("epool", 3)        # [128,1024] bf16: exp(S)
        sqpool = pool("sqpool", 3)      # squares for LN var
        sbig = pool("sbig", 2)          # [128,1024] f32: LN mean/rstd bcast
        rows = pool("rows", 2)          # [1,1024] f32: LN stat rows
        rrows = pool("rrows", 2)        # [1,512] f32: softmax recip rows
        recb = pool("recb", 2)          # [64,512] f32: recip bcast
        dallp = pool("dallp", 2)        # [16,512] f32: batched softmax denoms
        outfp = pool("outfp", 2)        # [128,256] f32: final out staging
        zpool = pool("zpool", 1)        # [16,*] bf16: lora z (1 slot per tag)
        lorab = pool("lorab", 1)        # [16,1024] bf16: lora B rows
        loraa = pool("loraa", 10)       # [128,16] bf16: lora A chunks
        smalls = pool("smalls", 1)      # [128,<=32] bias/g/b columns (per tag)
        onesp = pool("onesp", 1)
        bandp = pool("bandp", 1)
        bvp = pool("bvp", 1)            # [1,1024] bf16 v-bias rows

        # PSUM pools: 4 + 2 + 2 = 8 banks
        ps = pool("ps", 2, space="PSUM")   # [128,1024] f32: S tiles, LN stats, pr acc
        po = pool("po", 2, space="PSUM")   # [65..128,512] f32: attn out acc, pr acc
        pp = pool("pp", 2, space="PSUM")   # [128,512] f32: projections, z

        # ---- constants ----
        ones_c32 = onesp.tile([128, 1], f32, tag="oc32")
        nc.gpsimd.memset(ones_c32[:], 1.0)
        ones_c16 = onesp.tile([128, 1], bf16, tag="oc16")
        nc.gpsimd.memset(ones_c16[:], 1.0)
        ones_r16 = onesp.tile([1, 128], bf16, tag="or16")
        nc.gpsimd.memset(ones_r16[:], 1.0)
        ones_r32 = onesp.tile([1, 128], f32, tag="or32")
        nc.gpsimd.memset(ones_r32[:], 1.0)
        ones_row512 = onesp.tile([1, 512], bf16, tag="or512")
        nc.gpsimd.memset(ones_row512[:], 1.0)

        band_t = bandp.tile([128, 64], f32, tag="band")
        nc.sync.dma_start(band_t[:], band_d[:, :])
        # selector matrices: sel[mi] @ dall broadcasts head 2mi to rows 0:64
        # and head 2mi+1 to rows 64:128 (softmax denominator rescale)
        sel_t = []
        for mi in range(NT):
            st_ = smalls.tile([R, 128], f32, tag=f"sel{mi}", name=f"sel{mi}")
            nc.sync.dma_start(st_[:], sel_d[mi])
            sel_t.append(st_)
        eps_t = onesp.tile([1, 1], f32, tag="eps")
        nc.gpsimd.memset(eps_t[:], EPS)

        dma_rr = [0]
        def wdma(dst, src):
            # spread weight streaming across two DMA queues
            eng = (nc.sync, nc.gpsimd)[dma_rr[0] % 2]
            dma_rr[0] += 1
            eng.dma_start(dst, src)

        def load_percol(name, n=NT):
            t = smalls.tile([128, n], f32, tag=name)
            nc.sync.dma_start(t[:], bias_d[name].rearrange("(m p) -> p m", p=128))
            return t

        bias_t = {
            n: load_percol(n)
            for n in ["bq", "bk", "bsp", "bcq", "bcp", "bpr", "g1", "b1", "g2", "b2", "bck"]
        }
        bias_t["bfc"] = load_percol("bfc", 32)
        bv_t = bvp.tile([1, C], bf16, tag="bv")
        nc.sync.dma_start(bv_t[:], bvrow_d[:, :])
        bcv_t = bvp.tile([1, C], bf16, tag="bcv")
        nc.sync.dma_start(bcv_t[:], bcvrow_d[:, :])
        bck_row_t = bvp.tile([1, C], bf16, tag="bckr")
        nc.sync.dma_start(bck_row_t[:], bckrow_d[:, :])

        def load_lora_a(name):
            ts = []
            for k in range(NT):
                t = loraa.tile([128, R], bf16, tag="loraa")
                nc.sync.dma_start(t[:], a_d[name][k * 128:(k + 1) * 128, :])
                ts.append(t)
            return ts

        def load_lora_b(name):
            t = lorab.tile([R, C], bf16, tag="lorab")
            nc.sync.dma_start(t[:], b_d[name][:, :])
            return t

        # =============== helpers ===============
        def bcast_row(row, out_sb, Tn):
            # broadcast [1, Tn] f32 row to [128, Tn] SBUF via K=1 PE matmul
            for h in range(Tn // 512):
                sl = slice(h * 512, (h + 1) * 512)
                bp = pp.tile([128, 512], f32, tag="pp")
                nc.tensor.matmul(bp[:], ones_r32[:], row[0:1, sl], start=True, stop=True)
                nc.vector.tensor_copy(out_sb[:, sl], bp[:])

        def ln_stats_and_norm(src_tiles, g_col, b_col, out_tiles):
            """LayerNorm over channel (partition) dim; src 8x[128,512] f32 persistent."""
            mean_ps = ps.tile([1, TQ], f32, tag="ps")
            sq_ps = ps.tile([1, TQ], f32, tag="ps")
            for k in range(NT):
                xb = sqpool.tile([128, TQ], bf16, tag="sqo")
                nc.vector.tensor_copy(xb[:], src_tiles[k][:])
                sq = sqpool.tile([128, TQ], bf16, tag="sqo")
                nc.vector.tensor_mul(sq[:], xb[:], xb[:])
                nc.tensor.matmul(mean_ps[:], ones_c16[:], xb[:],
                                 start=(k == 0), stop=(k == NT - 1))
                nc.tensor.matmul(sq_ps[:], ones_c16[:], sq[:],
                                 start=(k == 0), stop=(k == NT - 1))
            mean_row = rows.tile([1, TQ], f32, tag="rows")
            rstd_row = rows.tile([1, TQ], f32, tag="rows")
            nc.vector.tensor_scalar_mul(mean_row[:], mean_ps[:], 1.0 / C)
            nc.vector.tensor_mul(rstd_row[:], mean_row[:], mean_row[:])
            nc.vector.scalar_tensor_tensor(rstd_row[:], sq_ps[:], 1.0 / C, rstd_row[:],
                                           op0=AL.mult, op1=AL.subtract)
            nc.scalar.activation(rstd_row[:], rstd_row[:], AF.Sqrt, bias=eps_t[:])
            nc.vector.reciprocal(rstd_row[:], rstd_row[:])
            mb = sbig.tile([128, TQ], f32, tag="sbig")
            rb = sbig.tile([128, TQ], f32, tag="sbig")
            bcast_row(mean_row, mb, TQ)
            bcast_row(rstd_row, rb, TQ)
            for k in range(NT):
                t1 = big32.tile([128, TQ], f32, tag="big32")
                nc.vector.tensor_sub(t1[:], src_tiles[k][:], mb[:])
                nc.vector.tensor_mul(t1[:], t1[:], rb[:])
                nc.scalar.activation(out_tiles[k][:], t1[:], AF.Identity,
                                     bias=b_col[:, k:k + 1], scale=g_col[:, k:k + 1])

        def compute_z(a_tiles, rhs_tiles, Tn, tag):
            """z^T = A-proj of activations: [16, Tn] bf16."""
            z_sb = zpool.tile([R, Tn], bf16, tag=tag)
            for h in range(Tn // 512):
                sl = slice(h * 512, (h + 1) * 512)
                zp = pp.tile([R, 512], f32, tag="pp")
                for k in range(NT):
                    nc.tensor.matmul(zp[:], a_tiles[k][:], rhs_tiles[k][:, sl],
                                     start=(k == 0), stop=(k == NT - 1))
                nc.vector.tensor_copy(z_sb[:, sl], zp[:])
            return z_sb

        def projT(wname, rhs_tiles, Tn, z_sb, bname, out_cb, pools=None,
                  bias_row_t=None):
            """out^T tiles via PE; lora + callback per (M-tile, t-half) psum."""
            if pools is None:
                pools = ((pp, "pp"),)
            b_t = load_lora_b(bname)
            pcnt = 0
            for mh in range(2):  # c_out halves of 512
                wts = []
                for k in range(NT):
                    wt = wpool.tile([128, 512], bf16, tag="wpool")
                    wdma(wt[:], w_d[wname][k * 128:(k + 1) * 128,
                                           mh * 512:(mh + 1) * 512])
                    wts.append(wt)
                for ml in range(4):
                    mi = mh * 4 + ml
                    for h in range(Tn // 512):
                        sl = slice(h * 512, (h + 1) * 512)
                        pl, ptag = pools[pcnt % len(pools)]
                        pcnt += 1
                        pt = pl.tile([128, 512], f32, tag=ptag)
                        for k in range(NT):
                            nc.tensor.matmul(pt[:], wts[k][:, ml * 128:(ml + 1) * 128],
                                             rhs_tiles[k][:, sl], start=(k == 0), stop=False)
                        if bias_row_t is not None:
                            nc.tensor.matmul(pt[:], bias_row_t[0:1, mi * 128:(mi + 1) * 128],
                                             ones_row512[:], start=False, stop=False)
                        nc.tensor.matmul(pt[:], b_t[:, mi * 128:(mi + 1) * 128],
                                         z_sb[:, sl], start=False, stop=True)
                        out_cb(mi, pt, h)

        def proj_V(wname, lhs_tiles, z_sb, bv_row_t, bname, v_tiles, pools=None):
            """V natural [t, d] with activations stationary; +lora +bias(ones-MM)."""
            if pools is None:
                pools = ((pp, "pp"),)
            b_t = load_lora_b(bname)
            pcnt = 0
            for dh in range(2):
                sl = slice(dh * 512, (dh + 1) * 512)
                wts = []
                for k in range(NT):
                    wt = wpool.tile([128, 512], bf16, tag="wpool")
                    wdma(wt[:], w_d[wname][k * 128:(k + 1) * 128, sl])
                    wts.append(wt)
                for tt in range(NT):
                    pl, ptag = pools[pcnt % len(pools)]
                    pcnt += 1
                    pt = pl.tile([128, 512], f32, tag=ptag)
                    for k in range(NT):
                        nc.tensor.matmul(pt[:], lhs_tiles[k][:, tt * 128:(tt + 1) * 128],
                                         wts[k][:], start=(k == 0), stop=False)
                    nc.tensor.matmul(pt[:], z_sb[:, tt * 128:(tt + 1) * 128],
                                     b_t[:, sl], start=False, stop=False)
                    nc.tensor.matmul(pt[:], ones_r16[:], bv_row_t[:, sl],
                                     start=False, stop=True)
                    dest = v_tiles[tt][:, dh * 520:(dh + 1) * 520]
                    dest = dest.rearrange("p (h d) -> p h d", d=65)[:, :, 0:64]
                    nc.vector.tensor_copy(dest, pt[:])

        def attention(q_tiles, k_tiles, v_tiles, o_tiles):
            # Block kj only matters for queries i >= 64*kj (strided parity
            # layout), so every matmul/exp runs on the live tq-subrange.
            dall = dallp.tile([R, 512], f32, tag="dallp")
            for h in range(H):
                mi, off = h // 2, 64 * (h % 2)
                op = po.tile([65, 512], f32, tag="po")
                for jp in range(4):
                    # two key blocks per psum pair tile; live score regions
                    # packed contiguously so one Exp covers both
                    kj0, kj1 = 2 * jp, 2 * jp + 1
                    q0, q1 = 64 * kj0, 64 * kj1
                    e1 = 512 + (512 - q1)
                    st = ps.tile([128, 1024], f32, tag="ps")
                    nc.tensor.matmul(
                        st[:, q0:512],
                        k_tiles[mi][off:off + 64, kj0 * 128:(kj0 + 1) * 128],
                        q_tiles[mi][off:off + 64, q0:512],
                        start=True, stop=True)
                    nc.tensor.matmul(
                        st[:, 512:e1],
                        k_tiles[mi][off:off + 64, kj1 * 128:(kj1 + 1) * 128],
                        q_tiles[mi][off:off + 64, q1:512],
                        start=True, stop=True)
                    # additive causal band on the diagonal-straddling queries
                    nc.vector.tensor_add(st[:, q0:q0 + 64], st[:, q0:q0 + 64], band_t[:])
                    nc.vector.tensor_add(st[:, 512:576], st[:, 512:576], band_t[:])
                    et = epool.tile([128, 1024], bf16, tag="epool")
                    nc.scalar.activation(et[:, q0:e1], st[:, q0:e1], AF.Exp)
                    nc.tensor.matmul(
                        op[:] if kj0 == 0 else op[:, q0:512],
                        v_tiles[kj0][:, 65 * h:65 * h + 65],
                        et[:, q0:512], start=(kj0 == 0), stop=False)
                    nc.tensor.matmul(
                        op[:, q1:512],
                        v_tiles[kj1][:, 65 * h:65 * h + 65],
                        et[:, 512:e1], start=False, stop=(kj1 == 7))
                # stash raw (unnormalized) head output + denominator row
                nc.vector.tensor_copy(o_tiles[mi][off:off + 64, :], op[0:64, :])
                rr = rrows.tile([1, 512], f32, tag="rrows")
                nc.vector.tensor_copy(rr[:], op[64:65, :])
                nc.sync.dma_start(dall[h:h + 1, :], rr[:])
            # one batched reciprocal for all 16 heads, then per-tile rescale
            nc.vector.reciprocal(dall[:], dall[:])
            for mi2 in range(NT):
                bp = pp.tile([128, 512], f32, tag="pp")
                nc.tensor.matmul(bp[:], sel_t[mi2][:], dall[:], start=True, stop=True)
                rbc = recb.tile([128, 512], f32, tag="recb")
                nc.vector.tensor_copy(rbc[:], bp[:])
                nc.vector.tensor_mul(o_tiles[mi2][:], o_tiles[mi2][:], rbc[:])

        # =============== phase 1: LN1 over full x (2-pass) + own x ===============
        mean_ps = ps.tile([1, T], f32, tag="ps")
        sq_ps = ps.tile([1, T], f32, tag="ps")
        for k in range(NT):
            xt = big32.tile([128, T], f32, tag="big32")
            nc.sync.dma_start(xt[:], xT_d[k * 128:(k + 1) * 128, :])
            xb = sqpool.tile([128, T], bf16, tag="sqf")
            nc.vector.tensor_copy(xb[:], xt[:])
            sq = sqpool.tile([128, T], bf16, tag="sqf")
            nc.vector.tensor_mul(sq[:], xb[:], xb[:])
            for hh in range(2):
                sl = slice(hh * 512, (hh + 1) * 512)
                nc.tensor.matmul(mean_ps[0:1, sl], ones_c16[:], xb[:, sl],
                                 start=(k == 0), stop=(k == NT - 1))
                nc.tensor.matmul(sq_ps[0:1, sl], ones_c16[:], sq[:, sl],
                                 start=(k == 0), stop=(k == NT - 1))
        mean_row = rows.tile([1, T], f32, tag="rows")
        rstd_row = rows.tile([1, T], f32, tag="rows")
        nc.vector.tensor_scalar_mul(mean_row[:], mean_ps[:], 1.0 / C)
        nc.vector.tensor_mul(rstd_row[:], mean_row[:], mean_row[:])
        nc.vector.scalar_tensor_tensor(rstd_row[:], sq_ps[:], 1.0 / C, rstd_row[:],
                                       op0=AL.mult, op1=AL.subtract)
        nc.scalar.activation(rstd_row[:], rstd_row[:], AF.Sqrt, bias=eps_t[:])
        nc.vector.reciprocal(rstd_row[:], rstd_row[:])
        mb_f = sbig.tile([128, T], f32, tag="sbig")
        rb_f = sbig.tile([128, T], f32, tag="sbig")
        bcast_row(mean_row, mb_f, T)
        bcast_row(rstd_row, rb_f, T)
        lnb = [acts.tile([128, T], bf16, tag="acts", name=f"lnb{i}") for i in range(NT)]
        for k in range(NT):
            xt = big32.tile([128, T], f32, tag="big32")
            nc.sync.dma_start(xt[:], xT_d[k * 128:(k + 1) * 128, :])
            nc.vector.tensor_sub(xt[:], xt[:], mb_f[:])
            nc.vector.tensor_mul(xt[:], xt[:], rb_f[:])
            nc.scalar.activation(lnb[k][:], xt[:], AF.Identity,
                                 bias=bias_t["b1"][:, k:k + 1], scale=bias_t["g1"][:, k:k + 1])

        # own-token x -> residual tiles + LN(own)
        resid = []
        for k in range(NT):
            rt = rpool.tile([128, TQ], f32, tag="rpool")
            nc.sync.dma_start(rt[:], xqT_d[k * 128:(k + 1) * 128, :])
            resid.append(rt)
        lnown = [lnsm.tile([128, TQ], bf16, tag="lnsm", name=f"lnown{i}") for i in range(NT)]
        ln_stats_and_norm(resid, bias_t["g1"], bias_t["b1"], lnown)

        # =============== phase 2: self qkv ===============
        a_sa_t = load_lora_a("a_sa")
        z_sa = compute_z(a_sa_t, lnb, T, "zbig")
        z_own = compute_z(a_sa_t, lnown, TQ, "zsm")

        qT = [qpool.tile([128, TQ], bf16, tag="qpool", name=f"qT{i}") for i in range(NT)]

        def q_cb(mi, pt, h):
            nc.scalar.activation(qT[mi][:], pt[:], AF.Identity,
                                 bias=bias_t["bq"][:, mi:mi + 1])

        projT("wq", lnown, TQ, z_own, "b_saq", q_cb, pools=((pp, "pp"), (po, "po"), (ps, "ps")))

        kT = [kpool.tile([128, T], bf16, tag="kpool", name=f"kT{i}") for i in range(NT)]

        def k_cb(mi, pt, h):
            nc.scalar.activation(kT[mi][:, h * 512:(h + 1) * 512], pt[:], AF.Identity,
                                 bias=bias_t["bk"][:, mi:mi + 1])

        projT("wk", lnb, T, z_sa, "b_sak", k_cb, pools=((pp, "pp"), (po, "po"), (ps, "ps")))

        vt = [vpool.tile([128, 1040], bf16, tag="vpool", name=f"vt{i}") for i in range(NT)]
        for tt in range(NT):
            nc.gpsimd.memset(vt[tt][:, 64:1040:65], 1.0)
        proj_V("wv", lnb, z_sa, bv_t, "b_sav", vt, pools=((pp, "pp"), (po, "po"), (ps, "ps")))

        # =============== phase 3: cross K (PE filler during self-attn) ===============
        fb = [acts.tile([128, T], bf16, tag="acts", name=f"fb{i}") for i in range(NT)]
        for k in range(NT):
            ft = big32.tile([128, T], f32, tag="big32")
            nc.gpsimd.dma_start(ft[:], fT_d[k * 128:(k + 1) * 128, :])
            nc.vector.tensor_copy(fb[k][:], ft[:])
        a_ck_t = load_lora_a("a_ck")
        z_ck = compute_z(a_ck_t, fb, T, "zbig2")
        k2T = [k2pool.tile([128, T], bf16, tag="k2pool", name=f"k2T{i}") for i in range(NT)]

        def k2_cb(mi, pt, h):
            # DVE drain (bias already folded in via ones-matmul) keeps the
            # Scalar engine free for self-attention Exp
            nc.vector.tensor_copy(k2T[mi][:, h * 512:(h + 1) * 512], pt[:])

        projT("wck", fb, T, z_ck, "b_ckk", k2_cb, bias_row_t=bck_row_t)

        # =============== phase 4: self attention ===============
        oT = [opool.tile([128, TQ], bf16, tag="opool", name=f"oT{i}") for i in range(NT)]
        attention(qT, kT, vt, oT)

        # =============== phase 5: cross V (reuses V slots) ===============
        v2t = [vpool.tile([128, 1040], bf16, tag="vpool", name=f"v2t{i}") for i in range(NT)]
        for tt in range(NT):
            nc.gpsimd.memset(v2t[tt][:, 64:1040:65], 1.0)
        proj_V("wcv", fb, z_ck, bcv_t, "b_ckv", v2t, pools=((pp, "pp"), (po, "po")))

        # =============== phase 6: self proj + residual ===============
        a_sp_t = load_lora_a("a_sp")
        z_sp = compute_z(a_sp_t, oT, TQ, "zsm")

        def sp_cb(mi, pt, h):
            nc.vector.scalar_tensor_tensor(resid[mi][:], pt[:], bias_t["bsp"][:, mi:mi + 1],
                                           resid[mi][:], op0=AL.add, op1=AL.add)

        projT("wsp", oT, TQ, z_sp, "b_sp", sp_cb, pools=((pp, "pp"), (po, "po"), (ps, "ps")))

        # =============== phase 7: LN1 on updated own tokens ===============
        ln1b = [lnsm.tile([128, TQ], bf16, tag="lnsm", name=f"ln1b{i}") for i in range(NT)]
        ln_stats_and_norm(resid, bias_t["g1"], bias_t["b1"], ln1b)

        # =============== phase 8: cross q ===============
        a_cq_t = load_lora_a("a_cq")
        z_cq = compute_z(a_cq_t, ln1b, TQ, "zsm")
        q2T = [qpool.tile([128, TQ], bf16, tag="qpool", name=f"q2T{i}") for i in range(NT)]

        def q2_cb(mi, pt, h):
            nc.scalar.activation(q2T[mi][:], pt[:], AF.Identity,
                                 bias=bias_t["bcq"][:, mi:mi + 1])

        projT("wcq", ln1b, TQ, z_cq, "b_cq", q2_cb, pools=((pp, "pp"), (po, "po"), (ps, "ps")))

        # =============== phase 9: cross attention ===============
        o2T = [opool.tile([128, TQ], bf16, tag="opool", name=f"o2T{i}") for i in range(NT)]
        attention(q2T, k2T, v2t, o2T)

        # =============== phase 10: cross proj + residual ===============
        a_cp_t = load_lora_a("a_cp")
        z_cp = compute_z(a_cp_t, o2T, TQ, "zsm")

        def cp_cb(mi, pt, h):
            nc.vector.scalar_tensor_tensor(resid[mi][:], pt[:], bias_t["bcp"][:, mi:mi + 1],
                                           resid[mi][:], op0=AL.add, op1=AL.add)

        projT("wcp", o2T, TQ, z_cp, "b_cp", cp_cb, pools=((pp, "pp"), (po, "po"), (ps, "ps")))

        # =============== phase 11: LN2 + MLP (per token-half) ===============
        ln2 = [lnsm.tile([128, TQ], bf16, tag="lnsm", name=f"ln2_{i}") for i in range(NT)]
        ln_stats_and_norm(resid, bias_t["g2"], bias_t["b2"], ln2)

        for th in range(2):
            tsl = slice(th * 256, (th + 1) * 256)
            m_sb = [None] * 32
            for grp in range(8):
                wts = []
                for k in range(NT):
                    wt = wpool.tile([128, 512], bf16, tag="wpool")
                    wdma(wt[:], w_d["wfc"][k * 128:(k + 1) * 128,
                                           grp * 512:(grp + 1) * 512])
                    wts.append(wt)
                for ml in range(4):
                    mi = grp * 4 + ml
                    pl, ptag = ((pp, "pp"), (ps, "ps"))[ml % 2]
                    pt = pl.tile([128, 256], f32, tag=ptag)
                    for k in range(NT):
                        nc.tensor.matmul(pt[:], wts[k][:, ml * 128:(ml + 1) * 128],
                                         ln2[k][:, tsl], start=(k == 0), stop=(k == NT - 1))
                    mt = mpool.tile([128, 256], bf16, tag="mpool")
                    nc.scalar.activation(mt[:], pt[:], AF.Gelu_apprx_tanh,
                                         bias=bias_t["bfc"][:, mi:mi + 1])
                    m_sb[mi] = mt

            for quad in range(2):
                qts = []
                for j in range(4):
                    p_ = ps if j < 2 else po
                    qts.append(p_.tile([128, 256], f32, tag="ps" if j < 2 else "po", name=f"prq{th}_{quad}_{j}"))
                for k in range(32):
                    wt = wpool.tile([128, 512], bf16, tag="wpool")
                    wdma(wt[:], w_d["wpr"][k * 128:(k + 1) * 128,
                                           quad * 512:(quad + 1) * 512])
                    for j in range(4):
                        nc.tensor.matmul(qts[j][:], wt[:, j * 128:(j + 1) * 128],
                                         m_sb[k][:], start=(k == 0), stop=(k == 31))
                for j in range(4):
                    mi = quad * 4 + j
                    of = outfp.tile([128, 256], f32, tag="outfp")
                    nc.vector.scalar_tensor_tensor(of[:], qts[j][:],
                                                   bias_t["bpr"][:, mi:mi + 1],
                                                   resid[mi][:, tsl],
                                                   op0=AL.add, op1=AL.add)
                    nc.sync.dma_start(outT_d[mi * 128:(mi + 1) * 128, tsl], of[:])

    nc.compile()
    return nc


def _get_program():
    global _PROG
    if _PROG is None:
        _PROG = _build_program()
    return _PROG


def _prep_shared(inputs):
    g = {}

    def bf(a):
        return np.ascontiguousarray(np.asarray(a, dtype=np.float32)).astype(BF)

    def f(a):
        return np.ascontiguousarray(np.asarray(a, dtype=np.float32))

    qw, kw, vw = (inputs["sa_qkv_w"][i * C:(i + 1) * C] for i in range(3))
    qb, kb, vb = (inputs["sa_qkv_b"][i * C:(i + 1) * C] for i in range(3))
    qlb, klb, vlb = (inputs["sa_qkv_lb"][i * C:(i + 1) * C] for i in range(3))
    inv = 1.0 / np.sqrt(DH)
    g["wq"] = bf(np.asarray(qw).T * inv)
    g["wk"] = bf(np.asarray(kw).T)
    g["wv"] = bf(np.asarray(vw).T)
    g["bq"] = f(np.asarray(qb) * inv)
    g["bk"] = f(kb)
    g["bv_row"] = bf(np.asarray(vb).reshape(1, C))
    g["a_sa"] = bf(np.asarray(inputs["sa_qkv_a"]).T)
    g["b_saq"] = bf(np.asarray(qlb).T * (SCALE * inv))
    g["b_sak"] = bf(np.asarray(klb).T * SCALE)
    g["b_sav"] = bf(np.asarray(vlb).T * SCALE)

    g["wsp"] = bf(np.asarray(inputs["sa_proj_w"]).T)
    g["bsp"] = f(inputs["sa_proj_b"])
    g["a_sp"] = bf(np.asarray(inputs["sa_proj_a"]).T)
    g["b_sp"] = bf(np.asarray(inputs["sa_proj_lb"]).T * SCALE)

    g["wcq"] = bf(np.asarray(inputs["ca_q_w"]).T * inv)
    g["bcq"] = f(np.asarray(inputs["ca_q_b"]) * inv)
    g["a_cq"] = bf(np.asarray(inputs["ca_q_a"]).T)
    g["b_cq"] = bf(np.asarray(inputs["ca_q_lb"]).T * (SCALE * inv))

    ckw, cvw = inputs["ca_kv_w"][0:C], inputs["ca_kv_w"][C:2 * C]
    ckb, cvb = inputs["ca_kv_b"][0:C], inputs["ca_kv_b"][C:2 * C]
    cklb, cvlb = inputs["ca_kv_lb"][0:C], inputs["ca_kv_lb"][C:2 * C]
    g["wck"] = bf(np.asarray(ckw).T)
    g["wcv"] = bf(np.asarray(cvw).T)
    g["bck"] = f(ckb)
    g["bck_row"] = bf(np.asarray(ckb).reshape(1, C))
    g["bcv_row"] = bf(np.asarray(cvb).reshape(1, C))
    g["a_ck"] = bf(np.asarray(inputs["ca_kv_a"]).T)
    g["b_ckk"] = bf(np.asarray(cklb).T * SCALE)
    g["b_ckv"] = bf(np.asarray(cvlb).T * SCALE)

    g["wcp"] = bf(np.asarray(inputs["ca_proj_w"]).T)
    g["bcp"] = f(inputs["ca_proj_b"])
    g["a_cp"] = bf(np.asarray(inputs["ca_proj_a"]).T)
    g["b_cp"] = bf(np.asarray(inputs["ca_proj_lb"]).T * SCALE)

    g["wfc"] = bf(np.asarray(inputs["fc_w"]).T)
    g["bfc"] = f(inputs["fc_b"])
    g["wpr"] = bf(np.asarray(inputs["pr_w"]).T)
    g["bpr"] = f(inputs["pr_b"])
    g["g1"] = f(inputs["ln1_g"])
    g["b1"] = f(inputs["ln1_b"])
    g["g2"] = f(inputs["ln2_g"])
    g["b2"] = f(inputs["ln2_b"])
    return g


def _make_in_maps(inputs):
    inputs = {k: np.asarray(v) for k, v in inputs.items()}
    x, feat = inputs["x"], inputs["feature"]
    B = x.shape[0]
    shared = _prep_shared(inputs)

    bands = []
    for p in range(2):
        jj = np.arange(128).reshape(128, 1)
        ii = np.arange(64).reshape(1, 64)
        bands.append(np.where(jj <= 2 * ii + p, 0.0, -10000.0).astype(np.float32))

    sel = np.zeros((NT, R, 128), np.float32)
    for mi in range(NT):
        sel[mi, 2 * mi, 0:64] = 1.0
        sel[mi, 2 * mi + 1, 64:128] = 1.0
    shared["sel"] = sel

    in_maps = []
    xTs = [np.ascontiguousarray(np.asarray(x[b]).T, dtype=np.float32) for b in range(B)]
    fTs = [np.ascontiguousarray(np.asarray(feat[b]).T, dtype=np.float32) for b in range(B)]
    for core in range(NCORES):
        b, p = core // 2, core % 2
        m = dict(shared)
        m["xT"] = xTs[b]
        m["xqT"] = np.ascontiguousarray(xTs[b][:, p::2])
        m["fT"] = fTs[b]
        m["band"] = bands[p]
        in_maps.append(m)
    return in_maps, B


def kernel(**inputs):
    from concourse.bass_utils import run_bass_kernel_spmd

    nc = _get_program()
    in_maps, B = _make_in_maps(inputs)
    res = run_bass_kernel_spmd(nc, in_maps, core_ids=list(range(NCORES)))
    out = np.zeros((B, T, C), np.float32)
    for core in range(NCORES):
        b, p = core // 2, core % 2
        out[b, p::2, :] = np.asarray(res.results[core]["outT"], dtype=np.float32).T
    return out



# revision 7
# speedup vs baseline: 1.3395x; 1.3395x over previous
"""Trainium2 Bass kernel for nn_Block_with_lora (dense transformer block).

Sharding: 8 cores = 4 batches x 2 token-parity shards. Each core computes
its 512 query tokens end-to-end; K/V projections over all 1024 tokens are
computed per-core (uniform SPMD program).

Key design points vs the naive version:
- LoRA is folded into the dense weights on the host (W_eff = W + s*B@A),
  so the kernel runs plain GEMMs. K-biases are dropped entirely (a
  per-query constant logit shift is softmax-invariant); V-biases are
  folded into the following projection's bias on the host.
- x is stored column-PERMUTED per core: own-parity tokens first, then the
  other parity. LN(x)[:, :512] then doubles as the query-side activations
  (no second LN pass), and self-attention keys split into two triangular
  512-blocks handled with one additive [128,128] band each.
- QK matmuls have K=64: the two heads of a head-pair sit in partition
  rows 0:64 / 64:128, so their QK matmuls are emitted adjacently and run
  concurrently in different PE row-groups (tile_position auto-derived).
- The softmax denominator rides the AV matmul as a 65th ones-column of V.
- rstd = exp(-0.5*ln(var+eps)) keeps Scalar on the natural_log_exp table
  set for the whole kernel (no sqrt-set thrash); GELU loads its set once.
- Cross-attention K/V projections are emitted as PE filler inside the
  (Scalar-bound) self-attention window.
"""

import sys

sys.path.insert(0, "/opt/trn_rl_repo")

import numpy as np
import ml_dtypes
from contextlib import ExitStack

BF = ml_dtypes.bfloat16

C = 1024
H = 16
DH = 64
T = 1024
TQ = 512
NT = 8  # C / 128
R = 16
EPS = 1e-5
NCORES = 8
SCALE = 1.0 / 16  # lora_alpha / r

_PROG = None


def _build_program():
    import concourse.bass as bass
    import concourse.tile as tile
    from concourse import mybir, bacc

    f32 = mybir.dt.float32
    bf16 = mybir.dt.bfloat16
    AF = mybir.ActivationFunctionType
    AL = mybir.AluOpType

    nc = bacc.Bacc("TRN2", target_bir_lowering=False, debug=False)

    def din(name, shape, dt=f32):
        return nc.dram_tensor(name, shape, dt, kind="ExternalInput").ap()

    xT_d = din("xT", [C, T])
    fT_d = din("fT", [C, T])
    band_d = din("band", [128, 64])
    sband_d = din("sband", [128, 256])

    w_d = {}
    for n in ["wq", "wk", "wv", "wsp", "wcq", "wck", "wcv", "wcp"]:
        w_d[n] = din(n, [C, C], bf16)
    w_d["wfc"] = din("wfc", [C, 4 * C], bf16)
    w_d["wpr"] = din("wpr", [4 * C, C], bf16)
    bias_d = {
        n: din(n, [C], f32)
        for n in ["bq", "bcq", "bsp", "bcp", "bpr", "g1", "b1", "g2", "b2"]
    }
    bias_d["bfc"] = din("bfc", [4 * C], f32)
    sel_d = din("sel", [NT, R, 128], f32)

    outT_d = nc.dram_tensor("outT", [C, TQ], f32, kind="ExternalOutput").ap()

    with tile.TileContext(nc) as tc, ExitStack() as ctx:

        def pool(name, bufs, space=None):
            kw = dict(name=name, bufs=bufs)
            if space:
                kw["space"] = space
            return ctx.enter_context(tc.tile_pool(**kw))

        # SBUF pools
        bigf = pool("bigf", 2)          # [128,1024] f32: x/f stream + LN temps
        acts = pool("acts", 16)         # [128,1024] bf16: lnb + fb, later MLP m
        lnsm = pool("lnsm", 8)          # [128,512] bf16: ln1b -> ln2
        qpool = pool("qpool", 8)        # [128,512] bf16: qT -> q2T
        kpool = pool("kpool", 8)        # [128,1024] bf16: kT (self)
        k2pool = pool("k2pool", 8)      # [128,1024] bf16: k2T (cross)
        vp1 = pool("vp1", 8)            # [128,1040] bf16: V self
        vp2 = pool("vp2", 8)            # [128,1040] bf16: V cross
        opool = pool("opool", 8)        # [128,512] bf16: oT -> o2T
        rpool = pool("rpool", 8)        # [128,512] f32: residual (persist)
        wpool = pool("wpool", 11)       # [128,512] bf16: weight chunks
        epool = pool("epool", 3)        # [128,1024] bf16: exp(S)
        sqpool = pool("sqpool", 2)      # squares for LN var
        sbig = pool("sbig", 2)          # [128,1024] f32: LN mean/rstd bcast
        rows = pool("rows", 2)          # [1,1024] f32: LN stat rows
        rrows = pool("rrows", 1)        # [1,512] f32: softmax denom rows
        recb = pool("recb", 2)          # [128,512] f32: recip bcast
        dallp = pool("dallp", 1)        # [16,512] f32: batched softmax denoms
        outfp = pool("outfp", 1)        # [128,512] f32: final out staging
        smalls = pool("smalls", 1)      # [128,<=32] bias/g/b columns (per tag)
        onesp = pool("onesp", 1)
        bandp = pool("bandp", 1)

        # PSUM pools: 4 + 2 + 2 = 8 banks
        ps = pool("ps", 2, space="PSUM")   # [128,1024] f32
        po = pool("po", 2, space="PSUM")   # [65..128,512] f32
        pp = pool("pp", 2, space="PSUM")   # [128,512] f32

        # ---- constants ----
        ones_c16 = onesp.tile([128, 1], bf16, tag="oc16")
        nc.gpsimd.memset(ones_c16[:], 1.0)
        ones_r32 = onesp.tile([1, 128], f32, tag="or32")
        nc.gpsimd.memset(ones_r32[:], 1.0)
        eps_t = onesp.tile([1, 1], f32, tag="eps")
        nc.gpsimd.memset(eps_t[:], EPS)

        band_t = bandp.tile([128, 64], f32, tag="band")
        nc.scalar.dma_start(band_t[:], band_d[:, :])
        sband_t = bandp.tile([128, 256], f32, tag="sband")
        nc.scalar.dma_start(sband_t[:], sband_d[:, :])
        sel_t = []
        for mi in range(NT):
            st_ = smalls.tile([R, 128], f32, tag=f"sel{mi}", name=f"sel{mi}")
            nc.scalar.dma_start(st_[:], sel_d[mi])
            sel_t.append(st_)

        dma_rr = [0]

        def wdma(dst, src):
            eng = (nc.sync, nc.gpsimd)[dma_rr[0] % 2]
            dma_rr[0] += 1
            eng.dma_start(dst, src)

        def load_percol(name, n=NT):
            t = smalls.tile([128, n], f32, tag=name)
            nc.scalar.dma_start(t[:], bias_d[name].rearrange("(m p) -> p m", p=128))
            return t

        bias_t = {
            n: load_percol(n)
            for n in ["bq", "bcq", "bsp", "bcp", "bpr", "g1", "b1", "g2", "b2"]
        }
        bias_t["bfc"] = load_percol("bfc", 32)

        # =============== helpers ===============
        def bcast_row(row, out_sb, Tn):
            # broadcast [1, Tn] f32 row to [128, Tn] SBUF via K=1 PE matmul
            for h in range(Tn // 512):
                sl = slice(h * 512, (h + 1) * 512)
                bp = pp.tile([128, 512], f32, tag="pp")
                nc.tensor.matmul(bp[:], ones_r32[:], row[0:1, sl], start=True, stop=True)
                nc.vector.tensor_copy(out_sb[:, sl], bp[:])

        def rstd_from_var(var_row):
            # in-place rstd = exp(-0.5 * ln(var + eps)); natural_log_exp set
            nc.scalar.activation(var_row[:], var_row[:], AF.Ln, bias=eps_t[:])
            nc.scalar.activation(var_row[:], var_row[:], AF.Exp, scale=-0.5)

        def ln_stats_and_norm(src_tiles, g_col, b_col, out_tiles):
            """LayerNorm over channel (partition) dim; src 8x[128,512] f32."""
            mean_ps = ps.tile([1, TQ], f32, tag="ps")
            sq_ps = ps.tile([1, TQ], f32, tag="ps")
            for k in range(NT):
                xb = sqpool.tile([128, TQ], bf16, tag="sqo")
                nc.vector.tensor_copy(xb[:], src_tiles[k][:])
                sq = sqpool.tile([128, TQ], bf16, tag="sqo")
                nc.vector.tensor_mul(sq[:], xb[:], xb[:])
                nc.tensor.matmul(mean_ps[:], ones_c16[:], xb[:],
                                 start=(k == 0), stop=(k == NT - 1))
                nc.tensor.matmul(sq_ps[:], ones_c16[:], sq[:],
                                 start=(k == 0), stop=(k == NT - 1))
            mean_row = rows.tile([1, TQ], f32, tag="rows")
            var_row = rows.tile([1, TQ], f32, tag="rows")
            nc.vector.tensor_scalar_mul(mean_row[:], mean_ps[:], 1.0 / C)
            nc.vector.tensor_mul(var_row[:], mean_row[:], mean_row[:])
            nc.vector.scalar_tensor_tensor(var_row[:], sq_ps[:], 1.0 / C, var_row[:],
                                           op0=AL.mult, op1=AL.subtract)
            rstd_from_var(var_row)
            mb = sbig.tile([128, TQ], f32, tag="sbig")
            rb = sbig.tile([128, TQ], f32, tag="sbig")
            bcast_row(mean_row, mb, TQ)
            bcast_row(var_row, rb, TQ)
            for k in range(NT):
                t1 = bigf.tile([128, TQ], f32, tag="bigf")
                nc.vector.tensor_sub(t1[:], src_tiles[k][:], mb[:])
                nc.vector.tensor_mul(t1[:], t1[:], rb[:])
                nc.scalar.activation(out_tiles[k][:], t1[:], AF.Identity,
                                     bias=b_col[:, k:k + 1], scale=g_col[:, k:k + 1])

        def projT(wname, rhs_tiles, Tn, out_cb, pools):
            """out^T = W^T @ rhs, tiles [128,512]; drain via out_cb(mi, pt, h)."""
            pcnt = 0
            for mh in range(2):
                wts = []
                for k in range(NT):
                    wt = wpool.tile([128, 512], bf16, tag="wpool")
                    wdma(wt[:], w_d[wname][k * 128:(k + 1) * 128,
                                           mh * 512:(mh + 1) * 512])
                    wts.append(wt)
                for ml in range(4):
                    mi = mh * 4 + ml
                    for h in range(Tn // 512):
                        sl = slice(h * 512, (h + 1) * 512)
                        pl, ptag = pools[pcnt % len(pools)]
                        pcnt += 1
                        pt = pl.tile([128, 512], f32, tag=ptag)
                        for k in range(NT):
                            nc.tensor.matmul(pt[:], wts[k][:, ml * 128:(ml + 1) * 128],
                                             rhs_tiles[k][:, sl],
                                             start=(k == 0), stop=(k == NT - 1))
                        out_cb(mi, pt, h)

        def projT_units(wname, rhs_tiles, Tn, out_cb, pools, skip_mh=0):
            """Same as projT but returns a list of closures (one per weight-load
            or psum-tile) for interleaved emission."""
            units = []
            state = {}
            pcnt = [0]

            def mk_load(mh):
                def f():
                    wts = []
                    for k in range(NT):
                        wt = wpool.tile([128, 512], bf16, tag="wpool")
                        wdma(wt[:], w_d[wname][k * 128:(k + 1) * 128,
                                               mh * 512:(mh + 1) * 512])
                        wts.append(wt)
                    state[mh] = wts
                return f

            def mk_tile(mh, ml, h):
                def f():
                    mi = mh * 4 + ml
                    sl = slice(h * 512, (h + 1) * 512)
                    pl, ptag = pools[pcnt[0] % len(pools)]
                    pcnt[0] += 1
                    pt = pl.tile([128, 512], f32, tag=ptag)
                    wts = state[mh]
                    for k in range(NT):
                        nc.tensor.matmul(pt[:], wts[k][:, ml * 128:(ml + 1) * 128],
                                         rhs_tiles[k][:, sl],
                                         start=(k == 0), stop=(k == NT - 1))
                    out_cb(mi, pt, h)
                return f

            for mh in range(skip_mh, 2):
                units.append(mk_load(mh))
                for ml in range(4):
                    for h in range(Tn // 512):
                        units.append(mk_tile(mh, ml, h))
            return units

        def proj_V_units(wname, lhs_tiles, v_tiles, pools):
            """V natural [t, d] with activations stationary, as closure units."""
            units = []
            state = {}
            pcnt = [0]

            def mk_load(dh):
                def f():
                    sl = slice(dh * 512, (dh + 1) * 512)
                    wts = []
                    for k in range(NT):
                        wt = wpool.tile([128, 512], bf16, tag="wpool")
                        wdma(wt[:], w_d[wname][k * 128:(k + 1) * 128, sl])
                        wts.append(wt)
                    state[dh] = wts
                return f

            def mk_tile(dh, tt):
                def f():
                    pl, ptag = pools[pcnt[0] % len(pools)]
                    pcnt[0] += 1
                    pt = pl.tile([128, 512], f32, tag=ptag)
                    wts = state[dh]
                    for k in range(NT):
                        nc.tensor.matmul(pt[:], lhs_tiles[k][:, tt * 128:(tt + 1) * 128],
                                         wts[k][:], start=(k == 0), stop=(k == NT - 1))
                    dest = v_tiles[tt][:, dh * 520:(dh + 1) * 520]
                    dest = dest.rearrange("p (h d) -> p h d", d=65)[:, :, 0:64]
                    nc.vector.tensor_copy(dest, pt[:])
                return f

            for dh in range(2):
                units.append(mk_load(dh))
                for tt in range(NT):
                    units.append(mk_tile(dh, tt))
            return units

        def attn_self_pair(mi, q_tiles, k_tiles, v_tiles, o_tiles, dall):
            """One head pair (heads 2mi, 2mi+1) of permuted-layout self-attn."""
            ops = [po.tile([65, 512], f32, tag="po", name=f"sop{mi}_{oi}")
                   for oi in range(2)]
            for jp in range(4):
                q0 = 128 * jp
                w = 512 - q0
                sts = []
                # QK for both heads emitted adjacently -> row-group concurrency
                for oi in range(2):
                    off = 64 * oi
                    st = ps.tile([128, 1024], f32, tag="ps")
                    nc.tensor.matmul(
                        st[:, q0:512],
                        k_tiles[mi][off:off + 64, q0:q0 + 128],
                        q_tiles[mi][off:off + 64, q0:512], start=True, stop=True)
                    nc.tensor.matmul(
                        st[:, 512:512 + w],
                        k_tiles[mi][off:off + 64, 512 + q0:512 + q0 + 128],
                        q_tiles[mi][off:off + 64, q0:512], start=True, stop=True)
                    sts.append(st)
                for oi in range(2):
                    st = sts[oi]
                    nc.vector.tensor_add(st[:, q0:q0 + 128], st[:, q0:q0 + 128],
                                         sband_t[:, 0:128])
                    nc.vector.tensor_add(st[:, 512:640], st[:, 512:640],
                                         sband_t[:, 128:256])
                    et = epool.tile([128, 1024], bf16, tag="epool")
                    nc.scalar.activation(et[:, q0:512 + w], st[:, q0:512 + w], AF.Exp)
                    h = 2 * mi + oi
                    op = ops[oi]
                    nc.tensor.matmul(
                        op[:] if jp == 0 else op[:, q0:512],
                        v_tiles[jp][:, 65 * h:65 * h + 65],
                        et[:, q0:512], start=(jp == 0), stop=False)
                    nc.tensor.matmul(
                        op[:, q0:512],
                        v_tiles[4 + jp][:, 65 * h:65 * h + 65],
                        et[:, 512:512 + w], start=False, stop=(jp == 3))
            for oi in range(2):
                h = 2 * mi + oi
                off = 64 * oi
                nc.vector.tensor_copy(o_tiles[mi][off:off + 64, :], ops[oi][0:64, :])
                rr = rrows.tile([1, 512], f32, tag="rrows")
                nc.vector.tensor_copy(rr[:], ops[oi][64:65, :])
                nc.sync.dma_start(dall[h:h + 1, :], rr[:])

        def attn_cross_pair(mi, q_tiles, k_tiles, v_tiles, o_tiles, dall):
            """One head pair of cross-attn (natural key order, strided queries)."""
            ops = [po.tile([65, 512], f32, tag="po", name=f"cop{mi}_{oi}")
                   for oi in range(2)]
            for jp in range(4):
                kj0, kj1 = 2 * jp, 2 * jp + 1
                q0, q1 = 64 * kj0, 64 * kj1
                e1 = 512 + (512 - q1)
                sts = []
                for oi in range(2):
                    off = 64 * oi
                    st = ps.tile([128, 1024], f32, tag="ps")
                    nc.tensor.matmul(
                        st[:, q0:512],
                        k_tiles[mi][off:off + 64, kj0 * 128:(kj0 + 1) * 128],
                        q_tiles[mi][off:off + 64, q0:512], start=True, stop=True)
                    nc.tensor.matmul(
                        st[:, 512:e1],
                        k_tiles[mi][off:off + 64, kj1 * 128:(kj1 + 1) * 128],
                        q_tiles[mi][off:off + 64, q1:512], start=True, stop=True)
                    sts.append(st)
                for oi in range(2):
                    st = sts[oi]
                    nc.vector.tensor_add(st[:, q0:q0 + 64], st[:, q0:q0 + 64],
                                         band_t[:])
                    nc.vector.tensor_add(st[:, 512:576], st[:, 512:576], band_t[:])
                    et = epool.tile([128, 1024], bf16, tag="epool")
                    nc.scalar.activation(et[:, q0:e1], st[:, q0:e1], AF.Exp)
                    h = 2 * mi + oi
                    op = ops[oi]
                    nc.tensor.matmul(
                        op[:] if kj0 == 0 else op[:, q0:512],
                        v_tiles[kj0][:, 65 * h:65 * h + 65],
                        et[:, q0:512], start=(kj0 == 0), stop=False)
                    nc.tensor.matmul(
                        op[:, q1:512],
                        v_tiles[kj1][:, 65 * h:65 * h + 65],
                        et[:, 512:e1], start=False, stop=(kj1 == 7))
            for oi in range(2):
                h = 2 * mi + oi
                off = 64 * oi
                nc.vector.tensor_copy(o_tiles[mi][off:off + 64, :], ops[oi][0:64, :])
                rr = rrows.tile([1, 512], f32, tag="rrows")
                nc.vector.tensor_copy(rr[:], ops[oi][64:65, :])
                nc.sync.dma_start(dall[h:h + 1, :], rr[:])

        def attn_epilogue(dall, o_tiles):
            nc.vector.reciprocal(dall[:], dall[:])
            for mi2 in range(NT):
                bp = pp.tile([128, 512], f32, tag="pp")
                nc.tensor.matmul(bp[:], sel_t[mi2][:], dall[:], start=True, stop=True)
                rbc = recb.tile([128, 512], f32, tag="recb")
                nc.vector.tensor_copy(rbc[:], bp[:])
                nc.vector.tensor_mul(o_tiles[mi2][:], o_tiles[mi2][:], rbc[:])

        # =============== phase 1: stream x,f; LN1 over full x ===============
        lnb = [acts.tile([128, T], bf16, tag="acts", name=f"lnb{i}") for i in range(NT)]
        fb = [acts.tile([128, T], bf16, tag="acts", name=f"fb{i}") for i in range(NT)]
        mean_ps = ps.tile([1, T], f32, tag="ps")
        sq_ps = ps.tile([1, T], f32, tag="ps")
        for k in range(NT):
            xt = bigf.tile([128, T], f32, tag="bigf")
            nc.sync.dma_start(xt[:], xT_d[k * 128:(k + 1) * 128, :])
            ft = bigf.tile([128, T], f32, tag="bigf2")
            nc.gpsimd.dma_start(ft[:], fT_d[k * 128:(k + 1) * 128, :])
            nc.vector.tensor_copy(fb[k][:], ft[:])
            nc.vector.tensor_copy(lnb[k][:], xt[:])  # raw x in bf16 (normalized later)
            sq = sqpool.tile([128, T], bf16, tag="sqf")
            nc.vector.tensor_mul(sq[:], lnb[k][:], lnb[k][:])
            for hh in range(2):
                sl = slice(hh * 512, (hh + 1) * 512)
                nc.tensor.matmul(mean_ps[0:1, sl], ones_c16[:], lnb[k][:, sl],
                                 start=(k == 0), stop=(k == NT - 1))
                nc.tensor.matmul(sq_ps[0:1, sl], ones_c16[:], sq[:, sl],
                                 start=(k == 0), stop=(k == NT - 1))
        # residual = own-parity raw x (f32)
        resid = []
        for k in range(NT):
            rt = rpool.tile([128, TQ], f32, tag="rpool")
            nc.scalar.dma_start(rt[:], xT_d[k * 128:(k + 1) * 128, 0:TQ])
            resid.append(rt)

        mean_row = rows.tile([1, T], f32, tag="rows")
        var_row = rows.tile([1, T], f32, tag="rows")
        nc.vector.tensor_scalar_mul(mean_row[:], mean_ps[:], 1.0 / C)
        nc.vector.tensor_mul(var_row[:], mean_row[:], mean_row[:])
        nc.vector.scalar_tensor_tensor(var_row[:], sq_ps[:], 1.0 / C, var_row[:],
                                       op0=AL.mult, op1=AL.subtract)
        rstd_from_var(var_row)
        mb_f = sbig.tile([128, T], f32, tag="sbig")
        rb_f = sbig.tile([128, T], f32, tag="sbig")
        bcast_row(mean_row, mb_f, T)
        bcast_row(var_row, rb_f, T)

        # cross-K first chunk as early PE fill (needs only fb)
        k2T = [k2pool.tile([128, T], bf16, tag="k2pool", name=f"k2T{i}")
               for i in range(NT)]

        def k2_cb(mi, pt, h):
            nc.vector.tensor_copy(k2T[mi][:, h * 512:(h + 1) * 512], pt[:])

        wck_units = projT_units("wck", fb, T, k2_cb, ((pp, "pp"),))
        # emit mh=0 (first 9 units) early
        for u in wck_units[:9]:
            u()

        # normalize lnb in place
        for k in range(NT):
            t1 = bigf.tile([128, T], f32, tag="bigf")
            nc.vector.tensor_sub(t1[:], lnb[k][:], mb_f[:])
            nc.vector.tensor_mul(t1[:], t1[:], rb_f[:])
            nc.scalar.activation(lnb[k][:], t1[:], AF.Identity,
                                 bias=bias_t["b1"][:, k:k + 1],
                                 scale=bias_t["g1"][:, k:k + 1])

        # =============== phase 2: self qkv ===============
        qT = [qpool.tile([128, TQ], bf16, tag="qpool", name=f"qT{i}")
              for i in range(NT)]

        def q_cb(mi, pt, h):
            nc.scalar.activation(qT[mi][:], pt[:], AF.Identity,
                                 bias=bias_t["bq"][:, mi:mi + 1])

        projT("wq", lnb, TQ, q_cb, ((pp, "pp"), (po, "po"), (ps, "ps")))

        kT = [kpool.tile([128, T], bf16, tag="kpool", name=f"kT{i}")
              for i in range(NT)]

        def k_cb(mi, pt, h):
            nc.vector.tensor_copy(kT[mi][:, h * 512:(h + 1) * 512], pt[:])

        projT("wk", lnb, T, k_cb, ((pp, "pp"), (po, "po"), (ps, "ps")))

        vt = [vp1.tile([128, 1040], bf16, tag="vp1", name=f"vt{i}")
              for i in range(NT)]
        for tt in range(NT):
            nc.gpsimd.memset(vt[tt][:, 64:1040:65], 1.0)
        for u in proj_V_units("wv", lnb, vt, ((pp, "pp"), (po, "po"), (ps, "ps"))):
            u()

        # =============== phase 3: self attention + fillers ===============
        v2t = [vp2.tile([128, 1040], bf16, tag="vp2", name=f"v2t{i}")
               for i in range(NT)]
        for tt in range(NT):
            nc.gpsimd.memset(v2t[tt][:, 64:1040:65], 1.0)
        fillers = wck_units[9:] + proj_V_units("wcv", fb, v2t, ((pp, "pp"),))
        oT = [opool.tile([128, TQ], bf16, tag="opool", name=f"oT{i}")
              for i in range(NT)]
        dall1 = dallp.tile([R, 512], f32, tag="dallp")
        fidx = 0
        for mi in range(NT):
            attn_self_pair(mi, qT, kT, vt, oT, dall1)
            take = 4 if mi < 4 else 3
            for _ in range(take):
                if fidx < len(fillers):
                    fillers[fidx]()
                    fidx += 1
        while fidx < len(fillers):
            fillers[fidx]()
            fidx += 1
        attn_epilogue(dall1, oT)

        # =============== phase 4: self proj + residual ===============
        def sp_cb(mi, pt, h):
            nc.vector.scalar_tensor_tensor(resid[mi][:], pt[:],
                                           bias_t["bsp"][:, mi:mi + 1],
                                           resid[mi][:], op0=AL.add, op1=AL.add)

        projT("wsp", oT, TQ, sp_cb, ((pp, "pp"), (po, "po"), (ps, "ps")))

        # =============== phase 5: LN1 on updated own tokens ===============
        ln1b = [lnsm.tile([128, TQ], bf16, tag="lnsm", name=f"ln1b{i}")
                for i in range(NT)]
        ln_stats_and_norm(resid, bias_t["g1"], bias_t["b1"], ln1b)

        # =============== phase 6: cross q ===============
        q2T = [qpool.tile([128, TQ], bf16, tag="qpool", name=f"q2T{i}")
               for i in range(NT)]

        def q2_cb(mi, pt, h):
            nc.scalar.activation(q2T[mi][:], pt[:], AF.Identity,
                                 bias=bias_t["bcq"][:, mi:mi + 1])

        projT("wcq", ln1b, TQ, q2_cb, ((pp, "pp"), (po, "po"), (ps, "ps")))

        # =============== phase 7: cross attention ===============
        o2T = [opool.tile([128, TQ], bf16, tag="opool", name=f"o2T{i}")
               for i in range(NT)]
        dall2 = dallp.tile([R, 512], f32, tag="dallp")
        for mi in range(NT):
            attn_cross_pair(mi, q2T, k2T, v2t, o2T, dall2)
        attn_epilogue(dall2, o2T)

        # =============== phase 8: cross proj + residual ===============
        def cp_cb(mi, pt, h):
            nc.vector.scalar_tensor_tensor(resid[mi][:], pt[:],
                                           bias_t["bcp"][:, mi:mi + 1],
                                           resid[mi][:], op0=AL.add, op1=AL.add)

        projT("wcp", o2T, TQ, cp_cb, ((pp, "pp"), (po, "po"), (ps, "ps")))

        # =============== phase 9: LN2 + MLP ===============
        ln2 = [lnsm.tile([128, TQ], bf16, tag="lnsm", name=f"ln2_{i}")
               for i in range(NT)]
        ln_stats_and_norm(resid, bias_t["g2"], bias_t["b2"], ln2)

        # m chunks stored 2-per-tile in the (now free) acts pool
        md = [acts.tile([128, 1024], bf16, tag="acts", name=f"md{i}")
              for i in range(16)]
        for grp in range(8):
            wts = []
            for k in range(NT):
                wt = wpool.tile([128, 512], bf16, tag="wpool")
                wdma(wt[:], w_d["wfc"][k * 128:(k + 1) * 128,
                                       grp * 512:(grp + 1) * 512])
                wts.append(wt)
            for ml in range(4):
                mi = grp * 4 + ml
                pl, ptag = ((pp, "pp"), (po, "po"))[ml % 2]
                pt = pl.tile([128, TQ], f32, tag=ptag)
                for k in range(NT):
                    nc.tensor.matmul(pt[:], wts[k][:, ml * 128:(ml + 1) * 128],
                                     ln2[k][:], start=(k == 0), stop=(k == NT - 1))
                dst = md[mi // 2][:, (mi % 2) * 512:(mi % 2 + 1) * 512]
                nc.scalar.activation(dst, pt[:], AF.Gelu_apprx_tanh,
                                     bias=bias_t["bfc"][:, mi:mi + 1])

        for quad in range(2):
            qts = []
            for j in range(4):
                p_ = ps if j < 2 else po
                qts.append(p_.tile([128, TQ], f32, tag="ps" if j < 2 else "po",
                                   name=f"prq{quad}_{j}"))
            for k in range(32):
                wt = wpool.tile([128, 512], bf16, tag="wpool")
                wdma(wt[:], w_d["wpr"][k * 128:(k + 1) * 128,
                                       quad * 512:(quad + 1) * 512])
                rhs = md[k // 2][:, (k % 2) * 512:(k % 2 + 1) * 512]
                for j in range(4):
                    nc.tensor.matmul(qts[j][:], wt[:, j * 128:(j + 1) * 128],
                                     rhs, start=(k == 0), stop=(k == 31))
            for j in range(4):
                mi = quad * 4 + j
                of = outfp.tile([128, TQ], f32, tag="outfp")
                nc.vector.scalar_tensor_tensor(of[:], qts[j][:],
                                               bias_t["bpr"][:, mi:mi + 1],
                                               resid[mi][:],
                                               op0=AL.add, op1=AL.add)
                nc.sync.dma_start(outT_d[mi * 128:(mi + 1) * 128, :], of[:])

    nc.compile()
    return nc


def _get_program():
    global _PROG
    if _PROG is None:
        _PROG = _build_program()
    return _PROG


def _prep_shared(inputs):
    g = {}

    def bf(a):
        return np.ascontiguousarray(np.asarray(a, dtype=np.float32)).astype(BF)

    def f(a):
        return np.ascontiguousarray(np.asarray(a, dtype=np.float32))

    def fold(w, lb, a):
        return np.asarray(w, np.float64) + SCALE * (
            np.asarray(lb, np.float64) @ np.asarray(a, np.float64))

    inv = 1.0 / np.sqrt(DH)

    qkv_eff = fold(inputs["sa_qkv_w"], inputs["sa_qkv_lb"], inputs["sa_qkv_a"])
    qw, kw, vw = (qkv_eff[i * C:(i + 1) * C] for i in range(3))
    qb, kb, vb = (np.asarray(inputs["sa_qkv_b"])[i * C:(i + 1) * C] for i in range(3))
    g["wq"] = bf(qw.T * inv)
    g["wk"] = bf(kw.T)
    g["wv"] = bf(vw.T)
    g["bq"] = f(qb * inv)
    # kb dropped: a per-query constant logit shift is softmax-invariant

    sp_eff = fold(inputs["sa_proj_w"], inputs["sa_proj_lb"], inputs["sa_proj_a"])
    g["wsp"] = bf(sp_eff.T)
    g["bsp"] = f(np.asarray(inputs["sa_proj_b"]) + vb @ sp_eff.T)

    cq_eff = fold(inputs["ca_q_w"], inputs["ca_q_lb"], inputs["ca_q_a"])
    g["wcq"] = bf(cq_eff.T * inv)
    g["bcq"] = f(np.asarray(inputs["ca_q_b"]) * inv)

    ckv_eff = fold(inputs["ca_kv_w"], inputs["ca_kv_lb"], inputs["ca_kv_a"])
    ckw, cvw = ckv_eff[0:C], ckv_eff[C:2 * C]
    cvb = np.asarray(inputs["ca_kv_b"])[C:2 * C]
    g["wck"] = bf(ckw.T)
    g["wcv"] = bf(cvw.T)

    cp_eff = fold(inputs["ca_proj_w"], inputs["ca_proj_lb"], inputs["ca_proj_a"])
    g["wcp"] = bf(cp_eff.T)
    g["bcp"] = f(np.asarray(inputs["ca_proj_b"]) + cvb @ cp_eff.T)

    g["wfc"] = bf(np.asarray(inputs["fc_w"]).T)
    g["bfc"] = f(inputs["fc_b"])
    g["wpr"] = bf(np.asarray(inputs["pr_w"]).T)
    g["bpr"] = f(inputs["pr_b"])
    g["g1"] = f(inputs["ln1_g"])
    g["b1"] = f(inputs["ln1_b"])
    g["g2"] = f(inputs["ln2_g"])
    g["b2"] = f(inputs["ln2_b"])

    sel = np.zeros((NT, R, 128), np.float32)
    for mi in range(NT):
        sel[mi, 2 * mi, 0:64] = 1.0
        sel[mi, 2 * mi + 1, 64:128] = 1.0
    g["sel"] = sel
    return g


def _make_in_maps(inputs):
    inputs = {k: np.asarray(v) for k, v in inputs.items()}
    x, feat = inputs["x"], inputs["feature"]
    B = x.shape[0]
    shared = _prep_shared(inputs)

    # cross-attention band (keys natural order, queries strided): [128, 64]
    bands = []
    for p in range(2):
        jj = np.arange(128).reshape(128, 1)
        ii = np.arange(64).reshape(1, 64)
        bands.append(np.where(jj <= 2 * ii + p, 0.0, -10000.0).astype(np.float32))

    # self-attention bands (permuted layout): [128, 256] = [A | B]
    rr_ = np.arange(128).reshape(128, 1)
    qq_ = np.arange(128).reshape(1, 128)
    bandA = np.where(rr_ <= qq_, 0.0, -10000.0).astype(np.float32)
    bandB_strict = np.where(rr_ < qq_, 0.0, -10000.0).astype(np.float32)
    sbands = [np.concatenate([bandA, bandB_strict], axis=1),
              np.concatenate([bandA, bandA], axis=1)]

    in_maps = []
    xTs = [np.ascontiguousarray(np.asarray(x[b]).T, dtype=np.float32)
           for b in range(B)]
    fTs = [np.ascontiguousarray(np.asarray(feat[b]).T, dtype=np.float32)
           for b in range(B)]
    for core in range(NCORES):
        b, p = core // 2, core % 2
        m = dict(shared)
        perm = np.concatenate([np.arange(p, T, 2), np.arange(1 - p, T, 2)])
        m["xT"] = np.ascontiguousarray(xTs[b][:, perm])
        m["fT"] = fTs[b]
        m["band"] = bands[p]
        m["sband"] = sbands[p]
        in_maps.append(m)
    return in_maps, B


def kernel(**inputs):
    from concourse.bass_utils import run_bass_kernel_spmd

    nc = _get_program()
    in_maps, B = _make_in_maps(inputs)
    res = run_bass_kernel_spmd(nc, in_maps, core_ids=list(range(NCORES)))
    out = np.zeros((B, T, C), np.float32)
    for core in range(NCORES):
        b, p = core // 2, core % 2
        out[b, p::2, :] = np.asarray(res.results[core]["outT"],
                                     dtype=np.float32).T
    return out
